# revision 1
# baseline (speedup 1.0000x reference)
"""BDC (Brownian Distance Covariance) pooling kernel for 8x Trainium2 cores.

kernel(x, t) -> [64, 205120] float32
  x: [64, 640, 100] float32, t: [1, 1] float32

Strategy: pure batch data-parallelism, 8 batches per NeuronCore. Per batch:
  - z_ij = d_i + d_j - 2 x_i.x_j via one augmented K=102 TensorE matmul
    (A = [xT; d; ones], B = [-2 xT; ones; d])
  - dcov = sqrt(exp(t) * z + eps) on ScalarE with fused row-sum accumulation
  - double-centering matrix (rm_i + rm_j - gm) via a K=2 TensorE matmul,
    applied with one VectorE tensor_tensor per upper-trapezoid tile
  - upper trapezoid (block r keeps cols >= 128 r) DMA'd out; host maps
    trapezoid -> packed upper-triangle rows.
"""
import os
from contextlib import ExitStack

import numpy as np

import concourse.bass as bass
import concourse.tile as tile
from concourse import bacc, mybir
from concourse.bass_utils import run_bass_kernel_spmd
from concourse.masks import make_identity

P = 128
M = 100
DIM = 640
NR = DIM // P
EPS = 1e-5
F32 = mybir.dt.float32
NCORES = 8
BPC = 8  # batches per core

TRAP_W = [DIM - P * r for r in range(NR)]
TRAP_OFF = [P * sum(TRAP_W[:r]) for r in range(NR)]
TRAP_SIZE = P * sum(TRAP_W)

LAST_EXEC_NS = None
_PROGRAM = None


def _build(nc, n_batch):
    x_dram = nc.dram_tensor("x", [n_batch, DIM, M], F32, kind="ExternalInput").ap()
    t_dram = nc.dram_tensor("t", [1, 1], F32, kind="ExternalInput").ap()
    out_dram = nc.dram_tensor("out", [n_batch, TRAP_SIZE], F32,
                              kind="ExternalOutput").ap()

    with tile.TileContext(nc) as tc, ExitStack() as ctx:
        const = ctx.enter_context(tc.tile_pool(name="const", bufs=1))
        sb = ctx.enter_context(tc.tile_pool(name="sb", bufs=2))
        dcv = ctx.enter_context(tc.tile_pool(name="dcv", bufs=2 * NR))
        ps_z = ctx.enter_context(tc.tile_pool(name="ps_z", bufs=2, space="PSUM"))
        ps_aux = ctx.enter_context(tc.tile_pool(name="ps_aux", bufs=1, space="PSUM"))
        ps_row = ctx.enter_context(tc.tile_pool(name="ps_row", bufs=1, space="PSUM"))

        ident = const.tile([P, P], F32)
        make_identity(nc, ident[:])
        ones_row = const.tile([1, DIM], F32)
        nc.vector.memset(ones_row[:], 1.0)
        ones100 = const.tile([M, 1], F32)
        nc.vector.memset(ones100[:], 1.0)
        eps_vec = const.tile([P, 1], F32)
        nc.vector.memset(eps_vec[:], EPS)
        t_vec = const.tile([P, 1], F32)
        nc.sync.dma_start(out=t_vec[:], in_=t_dram[0:1, 0:1].to_broadcast((P, 1)))
        s_vec = const.tile([P, 1], F32)
        nc.scalar.activation(s_vec[:], t_vec[:], mybir.ActivationFunctionType.Exp)
        L2 = const.tile([2, DIM], F32)
        nc.vector.memset(L2[0:1, :], 1.0)
        R2 = const.tile([2, DIM], F32)
        nc.sync.dma_start(out=R2[1:2, :], in_=ones_row[:])

        for b in range(n_batch):
            x_in = sb.tile([P, NR, M], F32, tag="x_in")
            nc.sync.dma_start(out=x_in[:],
                              in_=x_dram[b].rearrange("(r p) m -> p r m", p=P))

            A = sb.tile([102, DIM], F32, tag="A")
            for r in range(NR):
                tp = ps_aux.tile([M, P], F32, tag="aux")
                nc.tensor.transpose(tp[:], x_in[:, r, :], ident[:])
                nc.scalar.copy(A[0:M, r * P:(r + 1) * P], tp[:])

            sq = sb.tile([M, DIM], F32, tag="sq")
            nc.scalar.activation(sq[:], A[0:M, :],
                                 mybir.ActivationFunctionType.Square)
            d_ps = ps_row.tile([1, DIM], F32, tag="row")
            nc.tensor.matmul(d_ps[:, 0:512], ones100[:], sq[:, 0:512],
                             start=True, stop=True)
            nc.tensor.matmul(d_ps[:, 512:DIM], ones100[:], sq[:, 512:DIM],
                             start=True, stop=True)
            d_row = sb.tile([1, DIM], F32, tag="d_row")
            nc.scalar.copy(d_row[:], d_ps[:])
            nc.sync.dma_start(out=A[100:101, :], in_=d_row[:])
            nc.sync.dma_start(out=A[101:102, :], in_=ones_row[:])

            Bm = sb.tile([102, DIM], F32, tag="Bm")
            nc.vector.tensor_scalar_mul(Bm[0:M, :], A[0:M, :], -2.0)
            nc.sync.dma_start(out=Bm[100:101, :], in_=ones_row[:])
            nc.sync.dma_start(out=Bm[101:102, :], in_=d_row[:])

            rs5 = sb.tile([P, NR], F32, tag="rs5")
            dcov = []
            for r in range(NR):
                z_ps = ps_z.tile([P, DIM], F32, tag="z")
                nc.tensor.matmul(z_ps[:, 0:512], A[:, r * P:(r + 1) * P],
                                 Bm[:, 0:512], start=True, stop=True)
                nc.tensor.matmul(z_ps[:, 512:DIM], A[:, r * P:(r + 1) * P],
                                 Bm[:, 512:DIM], start=True, stop=True)
                dc = dcv.tile([P, DIM], F32, tag="dcov")
                nc.scalar.activation(dc[:], z_ps[:],
                                     mybir.ActivationFunctionType.Sqrt,
                                     bias=eps_vec[:], scale=s_vec[:],
                                     accum_out=rs5[:, r:r + 1])
                dcov.append(dc)

            rm_ps = ps_row.tile([1, DIM], F32, tag="row")
            for r in range(NR):
                nc.tensor.transpose(rm_ps[0:1, r * P:(r + 1) * P],
                                    rs5[:, r:r + 1], ident[:])
            rm_row = sb.tile([1, DIM], F32, tag="rm_row")
            gms = sb.tile([1, 1], F32, tag="gms")
            nc.scalar.activation(rm_row[:], rm_ps[:],
                                 mybir.ActivationFunctionType.Copy,
                                 scale=1.0 / DIM, accum_out=gms[:])
            gm = sb.tile([1, 1], F32, tag="gm")
            nc.vector.tensor_scalar_mul(gm[:], gms[:], 1.0 / DIM)
            nc.vector.tensor_scalar(R2[0:1, :], rm_row[:], gm[:], None,
                                    mybir.AluOpType.subtract)
            nc.sync.dma_start(out=L2[1:2, :], in_=rm_row[:])

            for r in range(NR):
                w = TRAP_W[r]
                c0 = r * P
                m2 = ps_aux.tile([P, DIM], F32, tag="aux")
                n0 = 0
                while n0 < w:
                    nn = min(512, w - n0)
                    nc.tensor.matmul(m2[:, n0:n0 + nn], L2[:, c0:c0 + P],
                                     R2[:, c0 + n0:c0 + n0 + nn],
                                     start=True, stop=True)
                    n0 += nn
                nc.vector.tensor_tensor(dcov[r][:, c0:DIM], dcov[r][:, c0:DIM],
                                        m2[:, 0:w], mybir.AluOpType.subtract)
                nc.sync.dma_start(
                    out=out_dram[b, TRAP_OFF[r]:TRAP_OFF[r] + P * w]
                        .rearrange("(p w) -> p w", p=P),
                    in_=dcov[r][:, c0:DIM],
                )
    return nc


def _get_program():
    global _PROGRAM
    if _PROGRAM is None:
        nc = bacc.Bacc("TRN2", target_bir_lowering=False, debug=False)
        _build(nc, BPC)
        nc.compile()
        _PROGRAM = nc
    return _PROGRAM


def _triu_index_map():
    iu_r, iu_c = np.triu_indices(DIM)
    r = iu_r // P
    off = np.array(TRAP_OFF)[r]
    w = np.array(TRAP_W)[r]
    return (off + (iu_r - r * P) * w + (iu_c - r * P)).astype(np.int64)


def kernel(x, t):
    global LAST_EXEC_NS
    x = np.ascontiguousarray(np.asarray(x, dtype=np.float32))
    t = np.ascontiguousarray(np.asarray(t, dtype=np.float32)).reshape(1, 1)
    B = x.shape[0]
    assert x.shape == (B, DIM, M) and B == NCORES * BPC

    nc = _get_program()
    in_maps = [{"x": x[c * BPC:(c + 1) * BPC], "t": t} for c in range(NCORES)]
    trace = os.environ.get("BDC_TRACE", "0") == "1"
    res = run_bass_kernel_spmd(nc, in_maps, list(range(NCORES)), trace=trace)
    LAST_EXEC_NS = res.exec_time_ns

    trap = np.concatenate([res.results[c]["out"] for c in range(NCORES)], axis=0)
    return trap[:, _triu_index_map()]
